# revision 8
# baseline (speedup 1.0000x reference)
"""Dual cross-attention kernel for Trainium2 (8 NeuronCores, SPMD).

Computes, per (b, h):
    scores1 = q1 @ k1.T ; scores2 = q2 @ k2.T          (contraction over E=64)
    A = tanh(scores1/8) * sigmoid(scores2/8)
    out = A @ v1                                        (contraction over S)

Sharding: B*H = 32 (b,h) pairs are split 4-per-core across 8 cores (pure
data parallelism; no collectives). The host reshapes full inputs to
[B*H, L, 64] and hands each core its [4, L, 64] shard.

On-chip dataflow per pair:
  - q1/q2 (and k1/k2) are DMA'd into an interleaved natural tile
    [128(l), 128(e2)] per l-tile (cols 0:64 = head1 E, 64:128 = head2 E),
    PE-transposed to q12T/k12T [128(e2), L/S] so the contraction dim E sits
    on partitions (rows 0:64 = head1, 64:128 = head2).
  - scoresT tiles [128(s), l_blk] are computed by two concurrent row-tiled
    K=64 matmuls (head1 on PE rows 0:63, head2 on rows 64:127), in float32r
    (full-rate 4-byte matmul at N>=256).
  - ScalarE applies tanh/sigmoid (scale=1/8 folded in), VectorE multiplies.
  - outT [64(d), l_blk] accumulates V.T @ A^T over s-tiles in PSUM,
    then is PE-transposed back to [l, d] and DMA'd out.
"""

import math
from contextlib import ExitStack

import numpy as np

import concourse.bass as bass
import concourse.mybir as mybir
import concourse.tile as tile
from concourse import bacc
from concourse.bass_utils import run_bass_kernel_spmd
from concourse.masks import make_identity

F32 = mybir.dt.float32
F32R = mybir.dt.float32r

B, L, S, H, E, D = 2, 2048, 2048, 16, 64, 64
N_CORES = 8
PAIRS_PER_CORE = (B * H) // N_CORES  # 4


def build_program(n_pairs=PAIRS_PER_CORE, l_sz=L, s_sz=S, use_f32r=True):
    """Build the single-core Bass/Tile program (SPMD across cores)."""
    nc = bacc.Bacc("TRN2", target_bir_lowering=False, debug=False)

    def din(name):
        return nc.dram_tensor(name, [n_pairs, l_sz if name[0] == "q" else s_sz, 64],
                              F32, kind="ExternalInput").ap()

    q1d, q2d, k1d, k2d = din("q1"), din("q2"), din("k1"), din("k2")
    vd = din("v1")
    outd = nc.dram_tensor("out", [n_pairs, l_sz, 64], F32, kind="ExternalOutput").ap()

    scale = 1.0 / math.sqrt(E)
    n_lt = l_sz // 128          # l-tiles per pair
    n_st = s_sz // 128          # s-tiles per pair
    l_blk = min(1024, l_sz)     # l-block processed per score tile
    n_lb = l_sz // l_blk
    n_mm = l_blk // 512 if l_blk >= 512 else 1   # matmuls (N<=512) per score tile
    mm_n = min(512, l_blk)

    MMDT = F32R if use_f32r else F32

    with tile.TileContext(nc) as tc, ExitStack() as ctx:
        const_p = ctx.enter_context(tc.tile_pool(name="const", bufs=1))
        nat_p = ctx.enter_context(tc.tile_pool(name="nat", bufs=2))
        qkT_p = ctx.enter_context(tc.tile_pool(name="qkT", bufs=4))
        v_p = ctx.enter_context(tc.tile_pool(name="v", bufs=2))
        act_p = ctx.enter_context(tc.tile_pool(name="act", bufs=4))
        a_p = ctx.enter_context(tc.tile_pool(name="aT", bufs=2))
        osb_p = ctx.enter_context(tc.tile_pool(name="osb", bufs=2))
        ps_tr = ctx.enter_context(tc.tile_pool(name="ptr", bufs=2, space="PSUM"))
        ps_s1 = ctx.enter_context(tc.tile_pool(name="ps1", bufs=1, space="PSUM"))
        ps_s2 = ctx.enter_context(tc.tile_pool(name="ps2", bufs=1, space="PSUM"))
        ps_o = ctx.enter_context(tc.tile_pool(name="pso", bufs=1, space="PSUM"))

        ident = const_p.tile([128, 128], F32)
        make_identity(nc, ident[:])

        def load_transposed(dram_a, dram_b, n_tiles):
            """Load two [n, 64] head tensors into a [128, n] E-on-partition
            tile (rows 0:64 = head a, 64:128 = head b)."""
            nat = nat_p.tile([128, n_tiles * 128], F32, tag="nat")
            natv = nat.rearrange("p (t e) -> p t e", e=128)
            nc.sync.dma_start(natv[:, :, 0:64],
                              dram_a.rearrange("(t p) e -> p t e", p=128))
            nc.sync.dma_start(natv[:, :, 64:128],
                              dram_b.rearrange("(t p) e -> p t e", p=128))
            dst = qkT_p.tile([128, n_tiles * 128], MMDT, tag="qkT")
            for t0 in range(0, n_tiles, 4):
                tn = min(4, n_tiles - t0)
                ps = ps_tr.tile([128, 512], F32, tag="ptr")
                for i in range(tn):
                    t = t0 + i
                    nc.tensor.transpose(ps[:, i * 128:(i + 1) * 128],
                                        nat[:, t * 128:(t + 1) * 128], ident[:])
                nc.vector.tensor_copy(dst[:, t0 * 128:(t0 + tn) * 128],
                                      ps[:, 0:tn * 128])
            return dst

        def load_v(dram_v):
            v_nat = v_p.tile([128, n_st * 64], F32, tag="v_nat")
            nc.sync.dma_start(v_nat.rearrange("p (t d) -> p t d", d=64),
                              dram_v.rearrange("(t p) d -> p t d", p=128))
            if not use_f32r:
                return v_nat
            # round to fp32r with a GpSimd cast-copy (keeps DVE/ACT free)
            v_sb = v_p.tile([128, n_st * 64], MMDT, tag="v")
            nc.gpsimd.tensor_copy(v_sb[:], v_nat[:])
            return v_sb

        for p in range(n_pairs):
            q12T = load_transposed(q1d[p], q2d[p], n_lt)
            k12T = load_transposed(k1d[p], k2d[p], n_st)
            v_sb = load_v(vd[p])

            for lb in range(n_lb):
                outT = ps_o.tile([64, l_blk], F32, tag="pso")
                for st in range(n_st):
                    s1 = ps_s1.tile([128, l_blk], F32, tag="ps1")
                    s2 = ps_s2.tile([128, l_blk], F32, tag="ps2")
                    ks = slice(st * 128, (st + 1) * 128)
                    for j in range(n_mm):
                        qs = slice(lb * l_blk + j * mm_n, lb * l_blk + (j + 1) * mm_n)
                        js = slice(j * mm_n, (j + 1) * mm_n)
                        nc.tensor.matmul(s1[:, js], k12T[0:64, ks],
                                         q12T[0:64, qs], start=True, stop=True)
                        nc.tensor.matmul(s2[:, js], k12T[64:128, ks],
                                         q12T[64:128, qs], start=True, stop=True)
                    t_sb = act_p.tile([128, l_blk], F32, tag="tanh")
                    nc.scalar.activation(t_sb[:], s1[:],
                                         mybir.ActivationFunctionType.Tanh,
                                         scale=scale)
                    g_sb = act_p.tile([128, l_blk], F32, tag="sig")
                    nc.scalar.activation(g_sb[:], s2[:],
                                         mybir.ActivationFunctionType.Sigmoid,
                                         scale=scale)
                    a_sb = a_p.tile([128, l_blk], MMDT, tag="aT")
                    nc.vector.tensor_mul(a_sb[:], t_sb[:], g_sb[:])
                    for j in range(n_mm):
                        js = slice(j * mm_n, (j + 1) * mm_n)
                        nc.tensor.matmul(outT[:, js],
                                         v_sb[:, st * 64:(st + 1) * 64],
                                         a_sb[:, js],
                                         start=(st == 0), stop=(st == n_st - 1))
                # epilogue: outT [64(d), l_blk] -> out [l, d]
                o_sb = osb_p.tile([64, l_blk], F32, tag="o_sb")
                nc.vector.tensor_copy(o_sb[:], outT[:])
                n_ch = l_blk // 128
                for c0 in range(0, n_ch, 8):
                    cn = min(8, n_ch - c0)
                    pso = ps_tr.tile([128, 512], F32, tag="ptr")
                    for i in range(cn):
                        c = c0 + i
                        nc.tensor.transpose(pso[:, i * 64:(i + 1) * 64],
                                            o_sb[:, c * 128:(c + 1) * 128],
                                            ident[0:64, 0:64])
                    ob = osb_p.tile([128, cn * 64], F32, tag="ob")
                    nc.vector.tensor_copy(ob[:], pso[:, 0:cn * 64])
                    lo = lb * l_blk + c0 * 128
                    nc.sync.dma_start(
                        outd[p, lo:lo + cn * 128, :].rearrange(
                            "(c p) d -> p c d", p=128),
                        ob.rearrange("p (c d) -> p c d", d=64))

    nc.compile()
    return nc


_PROG_CACHE = {}


def _get_program():
    key = (PAIRS_PER_CORE, L, S)
    if key not in _PROG_CACHE:
        _PROG_CACHE[key] = build_program()
    return _PROG_CACHE[key]


def _shard_inputs(q1, k1, v1, q2, k2):
    def shard(x):
        # [B, T, H, 64] -> [B*H, T, 64] -> per-core [PAIRS_PER_CORE, T, 64]
        xb = np.ascontiguousarray(
            np.asarray(x, dtype=np.float32).transpose(0, 2, 1, 3)
        ).reshape(B * H, -1, 64)
        return [np.ascontiguousarray(xb[c * PAIRS_PER_CORE:(c + 1) * PAIRS_PER_CORE])
                for c in range(N_CORES)]

    sh = {name: shard(x) for name, x in
          (("q1", q1), ("q2", q2), ("k1", k1), ("k2", k2), ("v1", v1))}
    return [{name: sh[name][c] for name in sh} for c in range(N_CORES)]


def _gather(results):
    out_bh = np.concatenate([results[c]["out"] for c in range(N_CORES)], axis=0)
    out = out_bh.reshape(B, H, L, D).transpose(0, 2, 1, 3)
    return np.ascontiguousarray(out.astype(np.float32))


def kernel(q1, k1, v1, q2, k2, v2, attn_mask=None, **_unused):
    """Full-input entry point: shards across 8 NeuronCores, returns [B,L,H,D]."""
    in_maps = _shard_inputs(q1, k1, v1, q2, k2)
    nc = _get_program()
    res = run_bass_kernel_spmd(nc, in_maps, list(range(N_CORES))).results
    return _gather(res)


def run_traced(q1, k1, v1, q2, k2, **kwargs):
    """Like kernel() but with NTFF profiling; returns (out, BassKernelResults)."""
    in_maps = _shard_inputs(q1, k1, v1, q2, k2)
    nc = _get_program()
    br = run_bass_kernel_spmd(nc, in_maps, list(range(N_CORES)), trace=True,
                              **kwargs)
    return _gather(br.results), br


# revision 9
# speedup vs baseline: 1.3940x; 1.3940x over previous
"""Dual cross-attention kernel for Trainium2 (8 NeuronCores, SPMD).

Computes, per (b, h):
    scores1 = q1 @ k1.T ; scores2 = q2 @ k2.T          (contraction over E=64)
    A = tanh(scores1/8) * sigmoid(scores2/8)
    out = A @ v1                                        (contraction over S)

Sharding: B*H = 32 (b,h) pairs are split 4-per-core across 8 cores (pure
data parallelism; no collectives). The host reshapes full inputs to
[B*H, L, 64] and hands each core its [4, L, 64] shard.

On-chip dataflow per pair:
  - q1/q2 (and k1/k2) are DMA'd into an interleaved natural tile
    [128(l), 128(e2)] per l-tile (cols 0:64 = head1 E, 64:128 = head2 E),
    PE-transposed to q12T/k12T [128(e2), L/S] so the contraction dim E sits
    on partitions (rows 0:64 = head1, 64:128 = head2).
  - scoresT tiles [128(s), l_blk] are computed by two concurrent row-tiled
    K=64 matmuls (head1 on PE rows 0:63, head2 on rows 64:127), in float32r
    (full-rate 4-byte matmul at N>=256).
  - ScalarE applies tanh/sigmoid (scale=1/8 folded in), VectorE multiplies.
  - outT [64(d), l_blk] accumulates V.T @ A^T over s-tiles in PSUM,
    then is PE-transposed back to [l, d] and DMA'd out.
"""

import math
from contextlib import ExitStack

import numpy as np

import concourse.bass as bass
import concourse.mybir as mybir
import concourse.tile as tile
from concourse import bacc
from concourse.bass_utils import run_bass_kernel_spmd
from concourse.masks import make_identity

F32 = mybir.dt.float32
F32R = mybir.dt.float32r

B, L, S, H, E, D = 2, 2048, 2048, 16, 64, 64
N_CORES = 8
PAIRS_PER_CORE = (B * H) // N_CORES  # 4


def build_program(n_pairs=PAIRS_PER_CORE, l_sz=L, s_sz=S, mm_dt=None):
    """Build the single-core Bass/Tile program (SPMD across cores)."""
    nc = bacc.Bacc("TRN2", target_bir_lowering=False, debug=False)

    def din(name):
        return nc.dram_tensor(name, [n_pairs, l_sz if name[0] == "q" else s_sz, 64],
                              F32, kind="ExternalInput").ap()

    q1d, q2d, k1d, k2d = din("q1"), din("q2"), din("k1"), din("k2")
    vd = din("v1")
    outd = nc.dram_tensor("out", [n_pairs, l_sz, 64], F32, kind="ExternalOutput").ap()

    scale = 1.0 / math.sqrt(E)
    n_lt = l_sz // 128          # l-tiles per pair
    n_st = s_sz // 128          # s-tiles per pair
    l_blk = min(1024, l_sz)     # l-block processed per score tile
    n_lb = l_sz // l_blk
    n_mm = l_blk // 512 if l_blk >= 512 else 1   # matmuls (N<=512) per score tile
    mm_n = min(512, l_blk)

    MMDT = mybir.dt.float16 if mm_dt is None else mm_dt

    with tile.TileContext(nc) as tc, ExitStack() as ctx:
        const_p = ctx.enter_context(tc.tile_pool(name="const", bufs=1))
        nat_p = ctx.enter_context(tc.tile_pool(name="nat", bufs=2))
        qkT_p = ctx.enter_context(tc.tile_pool(name="qkT", bufs=4))
        v_p = ctx.enter_context(tc.tile_pool(name="v", bufs=2))
        act_p = ctx.enter_context(tc.tile_pool(name="act", bufs=4))
        a_p = ctx.enter_context(tc.tile_pool(name="aT", bufs=2))
        osb_p = ctx.enter_context(tc.tile_pool(name="osb", bufs=2))
        ps_tr = ctx.enter_context(tc.tile_pool(name="ptr", bufs=2, space="PSUM"))
        ps_s1 = ctx.enter_context(tc.tile_pool(name="ps1", bufs=1, space="PSUM"))
        ps_s2 = ctx.enter_context(tc.tile_pool(name="ps2", bufs=1, space="PSUM"))
        ps_o = ctx.enter_context(tc.tile_pool(name="pso", bufs=1, space="PSUM"))

        ident = const_p.tile([128, 128], F32)
        make_identity(nc, ident[:])

        def load_transposed(dram_a, dram_b, n_tiles):
            """Load two [n, 64] head tensors into a [128, n] E-on-partition
            tile (rows 0:64 = head a, 64:128 = head b)."""
            nat = nat_p.tile([128, n_tiles * 128], F32, tag="nat")
            natv = nat.rearrange("p (t e) -> p t e", e=128)
            nc.sync.dma_start(natv[:, :, 0:64],
                              dram_a.rearrange("(t p) e -> p t e", p=128))
            nc.sync.dma_start(natv[:, :, 64:128],
                              dram_b.rearrange("(t p) e -> p t e", p=128))
            dst = qkT_p.tile([128, n_tiles * 128], MMDT, tag="qkT")
            for t0 in range(0, n_tiles, 4):
                tn = min(4, n_tiles - t0)
                ps = ps_tr.tile([128, 512], F32, tag="ptr")
                for i in range(tn):
                    t = t0 + i
                    nc.tensor.transpose(ps[:, i * 128:(i + 1) * 128],
                                        nat[:, t * 128:(t + 1) * 128], ident[:])
                nc.vector.tensor_copy(dst[:, t0 * 128:(t0 + tn) * 128],
                                      ps[:, 0:tn * 128])
            return dst

        def load_v(dram_v):
            v_nat = v_p.tile([128, n_st * 64], F32, tag="v_nat")
            nc.sync.dma_start(v_nat.rearrange("p (t d) -> p t d", d=64),
                              dram_v.rearrange("(t p) d -> p t d", p=128))
            if MMDT == F32:
                return v_nat
            # cast to matmul dtype on GpSimd (keeps DVE/ACT free)
            v_sb = v_p.tile([128, n_st * 64], MMDT, tag="v")
            nc.gpsimd.tensor_copy(v_sb[:], v_nat[:])
            return v_sb

        for p in range(n_pairs):
            q12T = load_transposed(q1d[p], q2d[p], n_lt)
            k12T = load_transposed(k1d[p], k2d[p], n_st)
            v_sb = load_v(vd[p])

            for lb in range(n_lb):
                outT = ps_o.tile([64, l_blk], F32, tag="pso")
                for st in range(n_st):
                    s1 = ps_s1.tile([128, l_blk], F32, tag="ps1")
                    s2 = ps_s2.tile([128, l_blk], F32, tag="ps2")
                    ks = slice(st * 128, (st + 1) * 128)
                    for j in range(n_mm):
                        qs = slice(lb * l_blk + j * mm_n, lb * l_blk + (j + 1) * mm_n)
                        js = slice(j * mm_n, (j + 1) * mm_n)
                        nc.tensor.matmul(s1[:, js], k12T[0:64, ks],
                                         q12T[0:64, qs], start=True, stop=True)
                        nc.tensor.matmul(s2[:, js], k12T[64:128, ks],
                                         q12T[64:128, qs], start=True, stop=True)
                    t_sb = act_p.tile([128, l_blk], MMDT, tag="tanh")
                    nc.scalar.activation(t_sb[:], s1[:],
                                         mybir.ActivationFunctionType.Tanh,
                                         scale=scale)
                    g_sb = act_p.tile([128, l_blk], MMDT, tag="sig")
                    nc.scalar.activation(g_sb[:], s2[:],
                                         mybir.ActivationFunctionType.Sigmoid,
                                         scale=scale)
                    a_sb = a_p.tile([128, l_blk], MMDT, tag="aT")
                    nc.vector.tensor_mul(a_sb[:], t_sb[:], g_sb[:])
                    for j in range(n_mm):
                        js = slice(j * mm_n, (j + 1) * mm_n)
                        nc.tensor.matmul(outT[:, js],
                                         v_sb[:, st * 64:(st + 1) * 64],
                                         a_sb[:, js],
                                         start=(st == 0), stop=(st == n_st - 1))
                # epilogue: outT [64(d), l_blk] -> out [l, d]
                o_sb = osb_p.tile([64, l_blk], F32, tag="o_sb")
                nc.vector.tensor_copy(o_sb[:], outT[:])
                n_ch = l_blk // 128
                for c0 in range(0, n_ch, 8):
                    cn = min(8, n_ch - c0)
                    pso = ps_tr.tile([128, 512], F32, tag="ptr")
                    for i in range(cn):
                        c = c0 + i
                        nc.tensor.transpose(pso[:, i * 64:(i + 1) * 64],
                                            o_sb[:, c * 128:(c + 1) * 128],
                                            ident[0:64, 0:64])
                    ob = osb_p.tile([128, cn * 64], F32, tag="ob")
                    nc.vector.tensor_copy(ob[:], pso[:, 0:cn * 64])
                    lo = lb * l_blk + c0 * 128
                    nc.sync.dma_start(
                        outd[p, lo:lo + cn * 128, :].rearrange(
                            "(c p) d -> p c d", p=128),
                        ob.rearrange("p (c d) -> p c d", d=64))

    nc.compile()
    return nc


_PROG_CACHE = {}


def _get_program():
    key = (PAIRS_PER_CORE, L, S)
    if key not in _PROG_CACHE:
        _PROG_CACHE[key] = build_program()
    return _PROG_CACHE[key]


def _shard_inputs(q1, k1, v1, q2, k2):
    def shard(x):
        # [B, T, H, 64] -> [B*H, T, 64] -> per-core [PAIRS_PER_CORE, T, 64]
        xb = np.ascontiguousarray(
            np.asarray(x, dtype=np.float32).transpose(0, 2, 1, 3)
        ).reshape(B * H, -1, 64)
        return [np.ascontiguousarray(xb[c * PAIRS_PER_CORE:(c + 1) * PAIRS_PER_CORE])
                for c in range(N_CORES)]

    sh = {name: shard(x) for name, x in
          (("q1", q1), ("q2", q2), ("k1", k1), ("k2", k2), ("v1", v1))}
    return [{name: sh[name][c] for name in sh} for c in range(N_CORES)]


def _gather(results):
    out_bh = np.concatenate([results[c]["out"] for c in range(N_CORES)], axis=0)
    out = out_bh.reshape(B, H, L, D).transpose(0, 2, 1, 3)
    return np.ascontiguousarray(out.astype(np.float32))


def kernel(q1, k1, v1, q2, k2, v2, attn_mask=None, **_unused):
    """Full-input entry point: shards across 8 NeuronCores, returns [B,L,H,D]."""
    in_maps = _shard_inputs(q1, k1, v1, q2, k2)
    nc = _get_program()
    res = run_bass_kernel_spmd(nc, in_maps, list(range(N_CORES))).results
    return _gather(res)


def run_traced(q1, k1, v1, q2, k2, **kwargs):
    """Like kernel() but with NTFF profiling; returns (out, BassKernelResults)."""
    in_maps = _shard_inputs(q1, k1, v1, q2, k2)
    nc = _get_program()
    br = run_bass_kernel_spmd(nc, in_maps, list(range(N_CORES)), trace=True,
                              **kwargs)
    return _gather(br.results), br


# revision 10
# speedup vs baseline: 1.5402x; 1.1049x over previous
"""Dual cross-attention kernel for Trainium2 (8 NeuronCores, SPMD).

Computes, per (b, h):
    scores1 = q1 @ k1.T ; scores2 = q2 @ k2.T          (contraction over E=64)
    A = tanh(scores1/8) * sigmoid(scores2/8)
    out = A @ v1                                        (contraction over S)

Sharding: B*H = 32 (b,h) pairs are split 4-per-core across 8 cores (pure
data parallelism; no collectives). The host reshapes full inputs to
[B*H, L, 64] and hands each core its [4, L, 64] shard.

On-chip dataflow per pair:
  - q1/q2 (and k1/k2) are DMA'd into an interleaved natural tile
    [128(l), 128(e2)] per l-tile (cols 0:64 = head1 E, 64:128 = head2 E),
    PE-transposed to q12T/k12T [128(e2), L/S] so the contraction dim E sits
    on partitions (rows 0:64 = head1, 64:128 = head2).
  - scoresT tiles [128(s), l_blk] are computed by two concurrent row-tiled
    K=64 matmuls (head1 on PE rows 0:63, head2 on rows 64:127), in float32r
    (full-rate 4-byte matmul at N>=256).
  - ScalarE applies tanh/sigmoid (scale=1/8 folded in), VectorE multiplies.
  - outT [64(d), l_blk] accumulates V.T @ A^T over s-tiles in PSUM,
    then is PE-transposed back to [l, d] and DMA'd out.
"""

import math
from contextlib import ExitStack

import numpy as np

import concourse.bass as bass
import concourse.mybir as mybir
import concourse.tile as tile
from concourse import bacc
from concourse.bass_utils import run_bass_kernel_spmd
from concourse.masks import make_identity

F32 = mybir.dt.float32
F32R = mybir.dt.float32r

B, L, S, H, E, D = 2, 2048, 2048, 16, 64, 64
N_CORES = 8
PAIRS_PER_CORE = (B * H) // N_CORES  # 4


def build_program(n_pairs=PAIRS_PER_CORE, l_sz=L, s_sz=S, mm_dt=None):
    """Build the single-core Bass/Tile program (SPMD across cores)."""
    nc = bacc.Bacc("TRN2", target_bir_lowering=False, debug=False)

    def din(name):
        return nc.dram_tensor(name, [n_pairs, l_sz if name[0] == "q" else s_sz, 64],
                              F32, kind="ExternalInput").ap()

    q1d, q2d, k1d, k2d = din("q1"), din("q2"), din("k1"), din("k2")
    vd = din("v1")
    outd = nc.dram_tensor("out", [n_pairs, l_sz, 64], F32, kind="ExternalOutput").ap()

    scale = 1.0 / math.sqrt(E)
    n_lt = l_sz // 128          # l-tiles per pair
    n_st = s_sz // 128          # s-tiles per pair
    l_blk = min(1024, l_sz)     # l-block processed per score tile
    n_lb = l_sz // l_blk
    n_mm = l_blk // 512 if l_blk >= 512 else 1   # matmuls (N<=512) per score tile
    mm_n = min(512, l_blk)

    MMDT = mybir.dt.float16 if mm_dt is None else mm_dt

    with tile.TileContext(nc) as tc, ExitStack() as ctx:
        const_p = ctx.enter_context(tc.tile_pool(name="const", bufs=1))
        nat_p = ctx.enter_context(tc.tile_pool(name="nat", bufs=2))
        qkT_p = ctx.enter_context(tc.tile_pool(name="qkT", bufs=4))
        v_p = ctx.enter_context(tc.tile_pool(name="v", bufs=2))
        act_p = ctx.enter_context(tc.tile_pool(name="act", bufs=4))
        a_p = ctx.enter_context(tc.tile_pool(name="aT", bufs=3))
        osb_p = ctx.enter_context(tc.tile_pool(name="osb", bufs=2))
        # one rotating score pool (3 x [128, l_blk] = 6 banks) + outT (2 banks)
        # = all 8 PSUM banks; input/output transposes tag-share the score pool
        sc_p = ctx.enter_context(tc.tile_pool(name="sc", bufs=3, space="PSUM"))
        ps_o = ctx.enter_context(tc.tile_pool(name="pso", bufs=1, space="PSUM"))

        ident = const_p.tile([128, 128], F32)
        make_identity(nc, ident[:])

        def load_transposed(dram_a, dram_b, n_tiles):
            """Load two [n, 64] head tensors into a [128, n] E-on-partition
            tile (rows 0:64 = head a, 64:128 = head b)."""
            nat = nat_p.tile([128, n_tiles * 128], F32, tag="nat")
            natv = nat.rearrange("p (t e) -> p t e", e=128)
            nc.sync.dma_start(natv[:, :, 0:64],
                              dram_a.rearrange("(t p) e -> p t e", p=128))
            nc.sync.dma_start(natv[:, :, 64:128],
                              dram_b.rearrange("(t p) e -> p t e", p=128))
            dst = qkT_p.tile([128, n_tiles * 128], MMDT, tag="qkT")
            for t0 in range(0, n_tiles, 4):
                tn = min(4, n_tiles - t0)
                ps = sc_p.tile([128, 512], F32, tag="sc")
                for i in range(tn):
                    t = t0 + i
                    nc.tensor.transpose(ps[:, i * 128:(i + 1) * 128],
                                        nat[:, t * 128:(t + 1) * 128], ident[:])
                nc.vector.tensor_copy(dst[:, t0 * 128:(t0 + tn) * 128],
                                      ps[:, 0:tn * 128])
            return dst

        def load_v(dram_v):
            v_nat = v_p.tile([128, n_st * 64], F32, tag="v_nat")
            nc.sync.dma_start(v_nat.rearrange("p (t d) -> p t d", d=64),
                              dram_v.rearrange("(t p) d -> p t d", p=128))
            if MMDT == F32:
                return v_nat
            # cast to matmul dtype on GpSimd (keeps DVE/ACT free)
            v_sb = v_p.tile([128, n_st * 64], MMDT, tag="v")
            nc.gpsimd.tensor_copy(v_sb[:], v_nat[:])
            return v_sb

        for p in range(n_pairs):
            q12T = load_transposed(q1d[p], q2d[p], n_lt)
            k12T = load_transposed(k1d[p], k2d[p], n_st)
            v_sb = load_v(vd[p])

            for lb in range(n_lb):
                outT = ps_o.tile([64, l_blk], F32, tag="pso")
                av_backlog = []
                for st in range(n_st):
                    s1 = sc_p.tile([128, l_blk], F32, tag="sc")
                    s2 = sc_p.tile([128, l_blk], F32, tag="sc")
                    ks = slice(st * 128, (st + 1) * 128)
                    for j in range(n_mm):
                        qs = slice(lb * l_blk + j * mm_n, lb * l_blk + (j + 1) * mm_n)
                        js = slice(j * mm_n, (j + 1) * mm_n)
                        nc.tensor.matmul(s1[:, js], k12T[0:64, ks],
                                         q12T[0:64, qs], start=True, stop=True)
                        nc.tensor.matmul(s2[:, js], k12T[64:128, ks],
                                         q12T[64:128, qs], start=True, stop=True)
                    # emit the previous iteration's AV matmuls here so the PE
                    # stream never waits on this iteration's ACT->DVE chain
                    if av_backlog:
                        av_backlog.pop(0)()
                    t_sb = act_p.tile([128, l_blk], MMDT, tag="tanh")
                    nc.scalar.activation(t_sb[:], s1[:],
                                         mybir.ActivationFunctionType.Tanh,
                                         scale=scale)
                    g_sb = act_p.tile([128, l_blk], MMDT, tag="sig")
                    nc.scalar.activation(g_sb[:], s2[:],
                                         mybir.ActivationFunctionType.Sigmoid,
                                         scale=scale)
                    a_sb = a_p.tile([128, l_blk], MMDT, tag="aT")
                    nc.vector.tensor_mul(a_sb[:], t_sb[:], g_sb[:])

                    def av(a_sb=a_sb, st=st):
                        for j in range(n_mm):
                            js = slice(j * mm_n, (j + 1) * mm_n)
                            nc.tensor.matmul(outT[:, js],
                                             v_sb[:, st * 64:(st + 1) * 64],
                                             a_sb[:, js],
                                             start=(st == 0),
                                             stop=(st == n_st - 1))
                    av_backlog.append(av)
                while av_backlog:
                    av_backlog.pop(0)()
                # epilogue: outT [64(d), l_blk] -> out [l, d]
                o_sb = osb_p.tile([64, l_blk], F32, tag="o_sb")
                nc.vector.tensor_copy(o_sb[:], outT[:])
                n_ch = l_blk // 128
                for c0 in range(0, n_ch, 8):
                    cn = min(8, n_ch - c0)
                    pso = sc_p.tile([128, 512], F32, tag="sc")
                    for i in range(cn):
                        c = c0 + i
                        nc.tensor.transpose(pso[:, i * 64:(i + 1) * 64],
                                            o_sb[:, c * 128:(c + 1) * 128],
                                            ident[0:64, 0:64])
                    ob = osb_p.tile([128, cn * 64], F32, tag="ob")
                    nc.vector.tensor_copy(ob[:], pso[:, 0:cn * 64])
                    lo = lb * l_blk + c0 * 128
                    nc.sync.dma_start(
                        outd[p, lo:lo + cn * 128, :].rearrange(
                            "(c p) d -> p c d", p=128),
                        ob.rearrange("p (c d) -> p c d", d=64))

    nc.compile()
    return nc


_PROG_CACHE = {}


def _get_program():
    key = (PAIRS_PER_CORE, L, S)
    if key not in _PROG_CACHE:
        _PROG_CACHE[key] = build_program()
    return _PROG_CACHE[key]


def _shard_inputs(q1, k1, v1, q2, k2):
    def shard(x):
        # [B, T, H, 64] -> [B*H, T, 64] -> per-core [PAIRS_PER_CORE, T, 64]
        xb = np.ascontiguousarray(
            np.asarray(x, dtype=np.float32).transpose(0, 2, 1, 3)
        ).reshape(B * H, -1, 64)
        return [np.ascontiguousarray(xb[c * PAIRS_PER_CORE:(c + 1) * PAIRS_PER_CORE])
                for c in range(N_CORES)]

    sh = {name: shard(x) for name, x in
          (("q1", q1), ("q2", q2), ("k1", k1), ("k2", k2), ("v1", v1))}
    return [{name: sh[name][c] for name in sh} for c in range(N_CORES)]


def _gather(results):
    out_bh = np.concatenate([results[c]["out"] for c in range(N_CORES)], axis=0)
    out = out_bh.reshape(B, H, L, D).transpose(0, 2, 1, 3)
    return np.ascontiguousarray(out.astype(np.float32))


def kernel(q1, k1, v1, q2, k2, v2, attn_mask=None, **_unused):
    """Full-input entry point: shards across 8 NeuronCores, returns [B,L,H,D]."""
    in_maps = _shard_inputs(q1, k1, v1, q2, k2)
    nc = _get_program()
    res = run_bass_kernel_spmd(nc, in_maps, list(range(N_CORES))).results
    return _gather(res)


def run_traced(q1, k1, v1, q2, k2, **kwargs):
    """Like kernel() but with NTFF profiling; returns (out, BassKernelResults)."""
    in_maps = _shard_inputs(q1, k1, v1, q2, k2)
    nc = _get_program()
    br = run_bass_kernel_spmd(nc, in_maps, list(range(N_CORES)), trace=True,
                              **kwargs)
    return _gather(br.results), br


# revision 13
# speedup vs baseline: 1.5557x; 1.0101x over previous
"""Dual cross-attention kernel for Trainium2 (8 NeuronCores, SPMD).

Computes, per (b, h):
    scores1 = q1 @ k1.T ; scores2 = q2 @ k2.T          (contraction over E=64)
    A = tanh(scores1/8) * sigmoid(scores2/8)
    out = A @ v1                                        (contraction over S)

Sharding: B*H = 32 (b,h) pairs are split 4-per-core across 8 cores (pure
data parallelism; no collectives). The host reshapes full inputs to
[B*H, L, 64] and hands each core its [4, L, 64] shard.

On-chip dataflow per pair:
  - q1/q2 (and k1/k2) are DMA'd into an interleaved natural tile
    [128(l), 128(e2)] per l-tile (cols 0:64 = head1 E, 64:128 = head2 E),
    PE-transposed to q12T/k12T [128(e2), L/S] so the contraction dim E sits
    on partitions (rows 0:64 = head1, 64:128 = head2).
  - scoresT tiles [128(s), l_blk] are computed by two concurrent row-tiled
    K=64 matmuls (head1 on PE rows 0:63, head2 on rows 64:127), in float32r
    (full-rate 4-byte matmul at N>=256).
  - ScalarE applies tanh/sigmoid (scale=1/8 folded in), VectorE multiplies.
  - outT [64(d), l_blk] accumulates V.T @ A^T over s-tiles in PSUM,
    then is PE-transposed back to [l, d] and DMA'd out.
"""

import math
from contextlib import ExitStack

import numpy as np

import concourse.bass as bass
import concourse.mybir as mybir
import concourse.tile as tile
from concourse import bacc
from concourse.bass_utils import run_bass_kernel_spmd
from concourse.masks import make_identity

F32 = mybir.dt.float32
F32R = mybir.dt.float32r

B, L, S, H, E, D = 2, 2048, 2048, 16, 64, 64
N_CORES = 8
PAIRS_PER_CORE = (B * H) // N_CORES  # 4


def build_program(n_pairs=PAIRS_PER_CORE, l_sz=L, s_sz=S, mm_dt=None):
    """Build the single-core Bass/Tile program (SPMD across cores)."""
    nc = bacc.Bacc("TRN2", target_bir_lowering=False, debug=False)

    def din(name):
        return nc.dram_tensor(name, [n_pairs, l_sz if name[0] == "q" else s_sz, 64],
                              F32, kind="ExternalInput").ap()

    q1d, q2d, k1d, k2d = din("q1"), din("q2"), din("k1"), din("k2")
    vd = din("v1")
    outd = nc.dram_tensor("out", [n_pairs, l_sz, 64], F32, kind="ExternalOutput").ap()

    scale = 1.0 / math.sqrt(E)
    n_lt = l_sz // 128          # l-tiles per pair
    n_st = s_sz // 128          # s-tiles per pair
    l_blk = min(1024, l_sz)     # l-block processed per score tile
    n_lb = l_sz // l_blk
    n_mm = l_blk // 512 if l_blk >= 512 else 1   # matmuls (N<=512) per score tile
    mm_n = min(512, l_blk)

    MMDT = mybir.dt.float16 if mm_dt is None else mm_dt

    with tile.TileContext(nc) as tc, ExitStack() as ctx:
        const_p = ctx.enter_context(tc.tile_pool(name="const", bufs=1))
        nat_p = ctx.enter_context(tc.tile_pool(name="nat", bufs=4))
        qkT_p = ctx.enter_context(tc.tile_pool(name="qkT", bufs=18))
        v_p = ctx.enter_context(tc.tile_pool(name="v", bufs=2))
        act_p = ctx.enter_context(tc.tile_pool(name="act", bufs=4))
        a_p = ctx.enter_context(tc.tile_pool(name="aT", bufs=3))
        osb_p = ctx.enter_context(tc.tile_pool(name="osb", bufs=2))
        # one rotating score pool (3 x [128, l_blk] = 6 banks) + outT (2 banks)
        # = all 8 PSUM banks; input/output transposes tag-share the score pool
        sc_p = ctx.enter_context(tc.tile_pool(name="sc", bufs=3, space="PSUM"))
        ps_o = ctx.enter_context(tc.tile_pool(name="pso", bufs=1, space="PSUM"))

        ident = const_p.tile([128, 128], F32)
        make_identity(nc, ident[:])

        def load_chunk(dram_a, dram_b, c0, tn):
            """Load l-tiles [c0, c0+tn) of two [n, 64] head tensors and
            transpose into a [128, tn*128] E-on-partition chunk."""
            nat = nat_p.tile([128, tn * 128], F32, tag="nat")
            natv = nat.rearrange("p (t e) -> p t e", e=128)
            nc.sync.dma_start(
                natv[:, :, 0:64],
                dram_a.rearrange("(t p) e -> p t e", p=128)[:, c0:c0 + tn, :])
            nc.sync.dma_start(
                natv[:, :, 64:128],
                dram_b.rearrange("(t p) e -> p t e", p=128)[:, c0:c0 + tn, :])
            ps = sc_p.tile([128, 512], F32, tag="sc")
            for i in range(tn):
                nc.tensor.transpose(ps[:, i * 128:(i + 1) * 128],
                                    nat[:, i * 128:(i + 1) * 128], ident[:])
            dst = qkT_p.tile([128, tn * 128], MMDT, tag="qkT")
            nc.vector.tensor_copy(dst[:], ps[:, 0:tn * 128])
            return dst

        def load_v(dram_v):
            v_nat = v_p.tile([128, n_st * 64], F32, tag="v_nat")
            nc.sync.dma_start(v_nat.rearrange("p (t d) -> p t d", d=64),
                              dram_v.rearrange("(t p) d -> p t d", p=128))
            if MMDT == F32:
                return v_nat
            # cast to matmul dtype on GpSimd (keeps DVE/ACT free)
            v_sb = v_p.tile([128, n_st * 64], MMDT, tag="v")
            nc.gpsimd.tensor_copy(v_sb[:], v_nat[:])
            return v_sb

        CHW = 4 * 128  # chunk width in columns (4 l-tiles)
        n_qch = (n_lt + 3) // 4
        n_kch = (n_st + 3) // 4

        class PairLoader:
            """Deferred-emission loader: chunks of q12T/k12T (+v) are emitted
            on demand or prefetched one at a time into the previous pair's
            main loop, so loads/transposes overlap steady-state compute."""

            def __init__(self, p):
                first_q = list(range(min((l_blk + CHW - 1) // CHW, n_qch)))
                rest_q = [c for c in range(n_qch) if c not in first_q]
                order = ([("q", c) for c in first_q] + [("k", 0), ("v", 0)]
                         + [("k", c) for c in range(1, n_kch)]
                         + [("q", c) for c in rest_q])
                self.plan = order
                self.p = p
                self.done = {}

            def _emit(self, key):
                kind, c = key
                if kind == "q":
                    tn = min(4, n_lt - 4 * c)
                    self.done[key] = load_chunk(q1d[self.p], q2d[self.p],
                                                4 * c, tn)
                elif kind == "k":
                    tn = min(4, n_st - 4 * c)
                    self.done[key] = load_chunk(k1d[self.p], k2d[self.p],
                                                4 * c, tn)
                else:
                    self.done[key] = load_v(vd[self.p])

            def require(self, key):
                while key not in self.done:
                    self._emit(self.plan.pop(0))

            def prefetch_one(self):
                if self.plan:
                    self._emit(self.plan.pop(0))

            def get(self, key):
                self.require(key)
                return self.done[key]

        loaders = [PairLoader(p) for p in range(n_pairs)]

        for p in range(n_pairs):
            ld = loaders[p]
            nxt = loaders[p + 1] if p + 1 < n_pairs else None

            for lb in range(n_lb):
                outT = ps_o.tile([64, l_blk], F32, tag="pso")
                av_backlog = []
                for st in range(n_st):
                    kch = ld.get(("k", st // 4))
                    kc = (st % 4) * 128
                    v_sb = ld.get(("v", 0))
                    s1 = sc_p.tile([128, l_blk], F32, tag="sc")
                    s2 = sc_p.tile([128, l_blk], F32, tag="sc")
                    for j in range(n_mm):
                        g = lb * l_blk + j * mm_n
                        qch = ld.get(("q", g // CHW))
                        qs = slice(g % CHW, g % CHW + mm_n)
                        js = slice(j * mm_n, (j + 1) * mm_n)
                        nc.tensor.matmul(s1[:, js], kch[0:64, kc:kc + 128],
                                         qch[0:64, qs], start=True, stop=True)
                        nc.tensor.matmul(s2[:, js], kch[64:128, kc:kc + 128],
                                         qch[64:128, qs], start=True, stop=True)
                    # emit the previous iteration's AV matmuls here so the PE
                    # stream never waits on this iteration's ACT->DVE chain
                    if av_backlog:
                        av_backlog.pop(0)()
                    t_sb = act_p.tile([128, l_blk], MMDT, tag="tanh")
                    nc.scalar.activation(t_sb[:], s1[:],
                                         mybir.ActivationFunctionType.Tanh,
                                         scale=scale)
                    g_sb = act_p.tile([128, l_blk], MMDT, tag="sig")
                    nc.scalar.activation(g_sb[:], s2[:],
                                         mybir.ActivationFunctionType.Sigmoid,
                                         scale=scale)
                    a_sb = a_p.tile([128, l_blk], MMDT, tag="aT")
                    nc.vector.tensor_mul(a_sb[:], t_sb[:], g_sb[:])

                    def av(a_sb=a_sb, st=st, v_sb=v_sb):
                        for j in range(n_mm):
                            js = slice(j * mm_n, (j + 1) * mm_n)
                            nc.tensor.matmul(outT[:, js],
                                             v_sb[:, st * 64:(st + 1) * 64],
                                             a_sb[:, js],
                                             start=(st == 0),
                                             stop=(st == n_st - 1))
                    av_backlog.append(av)
                    # steady prefetch of the next pair's input chunks
                    if nxt is not None and st % 3 == 2:
                        nxt.prefetch_one()
                while av_backlog:
                    av_backlog.pop(0)()
                # epilogue: outT [64(d), l_blk] -> out [l, d]
                o_sb = osb_p.tile([64, l_blk], F32, tag="o_sb")
                nc.vector.tensor_copy(o_sb[:], outT[:])
                n_ch = l_blk // 128
                for c0 in range(0, n_ch, 8):
                    cn = min(8, n_ch - c0)
                    pso = sc_p.tile([128, 512], F32, tag="sc")
                    for i in range(cn):
                        c = c0 + i
                        nc.tensor.transpose(pso[:, i * 64:(i + 1) * 64],
                                            o_sb[:, c * 128:(c + 1) * 128],
                                            ident[0:64, 0:64])
                    ob = osb_p.tile([128, cn * 64], F32, tag="ob")
                    nc.vector.tensor_copy(ob[:], pso[:, 0:cn * 64])
                    lo = lb * l_blk + c0 * 128
                    nc.sync.dma_start(
                        outd[p, lo:lo + cn * 128, :].rearrange(
                            "(c p) d -> p c d", p=128),
                        ob.rearrange("p (c d) -> p c d", d=64))

    nc.compile()
    return nc


_PROG_CACHE = {}


def _get_program():
    key = (PAIRS_PER_CORE, L, S)
    if key not in _PROG_CACHE:
        _PROG_CACHE[key] = build_program()
    return _PROG_CACHE[key]


def _shard_inputs(q1, k1, v1, q2, k2):
    def shard(x):
        # [B, T, H, 64] -> [B*H, T, 64] -> per-core [PAIRS_PER_CORE, T, 64]
        xb = np.ascontiguousarray(
            np.asarray(x, dtype=np.float32).transpose(0, 2, 1, 3)
        ).reshape(B * H, -1, 64)
        return [np.ascontiguousarray(xb[c * PAIRS_PER_CORE:(c + 1) * PAIRS_PER_CORE])
                for c in range(N_CORES)]

    sh = {name: shard(x) for name, x in
          (("q1", q1), ("q2", q2), ("k1", k1), ("k2", k2), ("v1", v1))}
    return [{name: sh[name][c] for name in sh} for c in range(N_CORES)]


def _gather(results):
    out_bh = np.concatenate([results[c]["out"] for c in range(N_CORES)], axis=0)
    out = out_bh.reshape(B, H, L, D).transpose(0, 2, 1, 3)
    return np.ascontiguousarray(out.astype(np.float32))


def kernel(q1, k1, v1, q2, k2, v2, attn_mask=None, **_unused):
    """Full-input entry point: shards across 8 NeuronCores, returns [B,L,H,D]."""
    in_maps = _shard_inputs(q1, k1, v1, q2, k2)
    nc = _get_program()
    res = run_bass_kernel_spmd(nc, in_maps, list(range(N_CORES))).results
    return _gather(res)


def run_traced(q1, k1, v1, q2, k2, **kwargs):
    """Like kernel() but with NTFF profiling; returns (out, BassKernelResults)."""
    in_maps = _shard_inputs(q1, k1, v1, q2, k2)
    nc = _get_program()
    br = run_bass_kernel_spmd(nc, in_maps, list(range(N_CORES)), trace=True,
                              **kwargs)
    return _gather(br.results), br


# revision 14
# speedup vs baseline: 1.6965x; 1.0905x over previous
"""Dual cross-attention kernel for Trainium2 (8 NeuronCores, SPMD).

Computes, per (b, h):
    scores1 = q1 @ k1.T ; scores2 = q2 @ k2.T          (contraction over E=64)
    A = tanh(scores1/8) * sigmoid(scores2/8)
    out = A @ v1                                        (contraction over S)

Sharding: B*H = 32 (b,h) pairs are split 4-per-core across 8 cores (pure
data parallelism; no collectives). The host reshapes full inputs to
[B*H, L, 64] and hands each core its [4, L, 64] shard.

On-chip dataflow per pair:
  - q1/q2 (and k1/k2) are DMA'd into an interleaved natural tile
    [128(l), 128(e2)] per l-tile (cols 0:64 = head1 E, 64:128 = head2 E),
    PE-transposed to q12T/k12T [128(e2), L/S] so the contraction dim E sits
    on partitions (rows 0:64 = head1, 64:128 = head2).
  - scoresT tiles [128(s), l_blk] are computed by two concurrent row-tiled
    K=64 matmuls (head1 on PE rows 0:63, head2 on rows 64:127), in float32r
    (full-rate 4-byte matmul at N>=256).
  - ScalarE applies tanh/sigmoid (scale=1/8 folded in), VectorE multiplies.
  - outT [64(d), l_blk] accumulates V.T @ A^T over s-tiles in PSUM,
    then is PE-transposed back to [l, d] and DMA'd out.
"""

import math
from contextlib import ExitStack

import numpy as np

import concourse.bass as bass
import concourse.mybir as mybir
import concourse.tile as tile
from concourse import bacc
from concourse.bass_utils import run_bass_kernel_spmd
from concourse.masks import make_identity

F32 = mybir.dt.float32
F32R = mybir.dt.float32r

B, L, S, H, E, D = 2, 2048, 2048, 16, 64, 64
N_CORES = 8
PAIRS_PER_CORE = (B * H) // N_CORES  # 4


def build_program(n_pairs=PAIRS_PER_CORE, l_sz=L, s_sz=S, mm_dt=None):
    """Build the single-core Bass/Tile program (SPMD across cores)."""
    nc = bacc.Bacc("TRN2", target_bir_lowering=False, debug=False)

    def din(name):
        return nc.dram_tensor(name, [n_pairs, l_sz if name[0] == "q" else s_sz, 64],
                              F32, kind="ExternalInput").ap()

    q1d, q2d, k1d, k2d = din("q1"), din("q2"), din("k1"), din("k2")
    vd = din("v1")
    outd = nc.dram_tensor("out", [n_pairs, l_sz, 64], F32, kind="ExternalOutput").ap()

    scale = 1.0 / math.sqrt(E)
    n_lt = l_sz // 128          # l-tiles per pair
    n_st = s_sz // 128          # s-tiles per pair
    l_blk = min(1024, l_sz)     # l-block processed per score tile
    n_lb = l_sz // l_blk
    n_mm = l_blk // 512 if l_blk >= 512 else 1   # matmuls (N<=512) per score tile
    mm_n = min(512, l_blk)

    MMDT = mybir.dt.float16 if mm_dt is None else mm_dt

    with tile.TileContext(nc) as tc, ExitStack() as ctx:
        const_p = ctx.enter_context(tc.tile_pool(name="const", bufs=1))
        nat_p = ctx.enter_context(tc.tile_pool(name="nat", bufs=4))
        qkT_p = ctx.enter_context(tc.tile_pool(name="qkT", bufs=18))
        v_p = ctx.enter_context(tc.tile_pool(name="v", bufs=2))
        act_p = ctx.enter_context(tc.tile_pool(name="act", bufs=4))
        a_p = ctx.enter_context(tc.tile_pool(name="aT", bufs=3))
        osb_p = ctx.enter_context(tc.tile_pool(name="osb", bufs=2))
        # one rotating score pool (3 x [128, l_blk] = 6 banks) + outT (2 banks)
        # = all 8 PSUM banks; input/output transposes tag-share the score pool
        sc_p = ctx.enter_context(tc.tile_pool(name="sc", bufs=3, space="PSUM"))
        ps_o = ctx.enter_context(tc.tile_pool(name="pso", bufs=1, space="PSUM"))

        ident = const_p.tile([128, 128], F32)
        make_identity(nc, ident[:])

        def load_chunk(dram_a, dram_b, c0, tn):
            """Load l-tiles [c0, c0+tn) of two [n, 64] head tensors and
            transpose into a [128, tn*128] E-on-partition chunk."""
            nat = nat_p.tile([128, tn * 128], F32, tag="nat")
            natv = nat.rearrange("p (t e) -> p t e", e=128)
            nc.sync.dma_start(
                natv[:, :, 0:64],
                dram_a.rearrange("(t p) e -> p t e", p=128)[:, c0:c0 + tn, :])
            nc.sync.dma_start(
                natv[:, :, 64:128],
                dram_b.rearrange("(t p) e -> p t e", p=128)[:, c0:c0 + tn, :])
            ps = sc_p.tile([128, 512], F32, tag="sc")
            for i in range(tn):
                nc.tensor.transpose(ps[:, i * 128:(i + 1) * 128],
                                    nat[:, i * 128:(i + 1) * 128], ident[:])
            dst = qkT_p.tile([128, tn * 128], MMDT, tag="qkT")
            nc.vector.tensor_copy(dst[:], ps[:, 0:tn * 128])
            return dst

        def load_v(dram_v):
            v_nat = v_p.tile([128, n_st * 64], F32, tag="v_nat")
            nc.sync.dma_start(v_nat.rearrange("p (t d) -> p t d", d=64),
                              dram_v.rearrange("(t p) d -> p t d", p=128))
            if MMDT == F32:
                return v_nat
            # cast to matmul dtype on GpSimd (keeps DVE/ACT free)
            v_sb = v_p.tile([128, n_st * 64], MMDT, tag="v")
            nc.gpsimd.tensor_copy(v_sb[:], v_nat[:])
            return v_sb

        CHW = 4 * 128  # chunk width in columns (4 l-tiles)
        n_qch = (n_lt + 3) // 4
        n_kch = (n_st + 3) // 4

        class PairLoader:
            """Deferred-emission loader: chunks of q12T/k12T (+v) are emitted
            on demand or prefetched one at a time into the previous pair's
            main loop, so loads/transposes overlap steady-state compute."""

            def __init__(self, p):
                first_q = list(range(min((l_blk + CHW - 1) // CHW, n_qch)))
                rest_q = [c for c in range(n_qch) if c not in first_q]
                order = ([("q", c) for c in first_q] + [("k", 0), ("v", 0)]
                         + [("k", c) for c in range(1, n_kch)]
                         + [("q", c) for c in rest_q])
                self.plan = order
                self.p = p
                self.done = {}

            def _emit(self, key):
                kind, c = key
                if kind == "q":
                    tn = min(4, n_lt - 4 * c)
                    self.done[key] = load_chunk(q1d[self.p], q2d[self.p],
                                                4 * c, tn)
                elif kind == "k":
                    tn = min(4, n_st - 4 * c)
                    self.done[key] = load_chunk(k1d[self.p], k2d[self.p],
                                                4 * c, tn)
                else:
                    self.done[key] = load_v(vd[self.p])

            def require(self, key):
                while key not in self.done:
                    self._emit(self.plan.pop(0))

            def prefetch_one(self):
                if self.plan:
                    self._emit(self.plan.pop(0))

            def get(self, key):
                self.require(key)
                return self.done[key]

        loaders = [PairLoader(p) for p in range(n_pairs)]

        # Deferred work queues: AV matmuls run one iteration late, and each
        # l-block's epilogue (outT readout/transpose/store) is drained inside
        # the NEXT block's iterations — so neither ever stalls the PE stream
        # ahead of the next block's score matmuls.
        av_backlog = []
        epi_backlog = []

        def pop_backlogs():
            if av_backlog:
                av_backlog.pop(0)()
            if epi_backlog:
                epi_backlog.pop(0)()

        def make_epilogue(outT, p, lb):
            def epi():
                # outT [64(d), l_blk] -> out [l, d]
                o_sb = osb_p.tile([64, l_blk], F32, tag="o_sb")
                nc.vector.tensor_copy(o_sb[:], outT[:])
                n_ch = l_blk // 128
                for c0 in range(0, n_ch, 8):
                    cn = min(8, n_ch - c0)
                    pso = sc_p.tile([128, 512], F32, tag="sc")
                    for i in range(cn):
                        c = c0 + i
                        nc.tensor.transpose(pso[:, i * 64:(i + 1) * 64],
                                            o_sb[:, c * 128:(c + 1) * 128],
                                            ident[0:64, 0:64])
                    ob = osb_p.tile([128, cn * 64], F32, tag="ob")
                    nc.vector.tensor_copy(ob[:], pso[:, 0:cn * 64])
                    lo = lb * l_blk + c0 * 128
                    nc.sync.dma_start(
                        outd[p, lo:lo + cn * 128, :].rearrange(
                            "(c p) d -> p c d", p=128),
                        ob.rearrange("p (c d) -> p c d", d=64))
            return epi

        for p in range(n_pairs):
            ld = loaders[p]
            nxt = loaders[p + 1] if p + 1 < n_pairs else None

            for lb in range(n_lb):
                outT = ps_o.tile([64, l_blk], F32, tag="pso")
                for st in range(n_st):
                    kch = ld.get(("k", st // 4))
                    kc = (st % 4) * 128
                    v_sb = ld.get(("v", 0))
                    s1 = sc_p.tile([128, l_blk], F32, tag="sc")
                    s2 = sc_p.tile([128, l_blk], F32, tag="sc")
                    for j in range(n_mm):
                        g = lb * l_blk + j * mm_n
                        qch = ld.get(("q", g // CHW))
                        qs = slice(g % CHW, g % CHW + mm_n)
                        js = slice(j * mm_n, (j + 1) * mm_n)
                        nc.tensor.matmul(s1[:, js], kch[0:64, kc:kc + 128],
                                         qch[0:64, qs], start=True, stop=True)
                        nc.tensor.matmul(s2[:, js], kch[64:128, kc:kc + 128],
                                         qch[64:128, qs], start=True, stop=True)
                    pop_backlogs()
                    t_sb = act_p.tile([128, l_blk], MMDT, tag="tanh")
                    nc.scalar.activation(t_sb[:], s1[:],
                                         mybir.ActivationFunctionType.Tanh,
                                         scale=scale)
                    g_sb = act_p.tile([128, l_blk], MMDT, tag="sig")
                    nc.scalar.activation(g_sb[:], s2[:],
                                         mybir.ActivationFunctionType.Sigmoid,
                                         scale=scale)
                    a_sb = a_p.tile([128, l_blk], MMDT, tag="aT")
                    nc.vector.tensor_mul(a_sb[:], t_sb[:], g_sb[:])

                    def av(a_sb=a_sb, st=st, v_sb=v_sb, outT=outT):
                        for j in range(n_mm):
                            js = slice(j * mm_n, (j + 1) * mm_n)
                            nc.tensor.matmul(outT[:, js],
                                             v_sb[:, st * 64:(st + 1) * 64],
                                             a_sb[:, js],
                                             start=(st == 0),
                                             stop=(st == n_st - 1))
                    av_backlog.append(av)
                    # steady prefetch of the next pair's input chunks
                    if nxt is not None and st % 3 == 2:
                        nxt.prefetch_one()
                epi_backlog.append(make_epilogue(outT, p, lb))

        while av_backlog or epi_backlog:
            pop_backlogs()

    nc.compile()
    return nc


_PROG_CACHE = {}


def _get_program():
    key = (PAIRS_PER_CORE, L, S)
    if key not in _PROG_CACHE:
        _PROG_CACHE[key] = build_program()
    return _PROG_CACHE[key]


def _shard_inputs(q1, k1, v1, q2, k2):
    def shard(x):
        # [B, T, H, 64] -> [B*H, T, 64] -> per-core [PAIRS_PER_CORE, T, 64]
        xb = np.ascontiguousarray(
            np.asarray(x, dtype=np.float32).transpose(0, 2, 1, 3)
        ).reshape(B * H, -1, 64)
        return [np.ascontiguousarray(xb[c * PAIRS_PER_CORE:(c + 1) * PAIRS_PER_CORE])
                for c in range(N_CORES)]

    sh = {name: shard(x) for name, x in
          (("q1", q1), ("q2", q2), ("k1", k1), ("k2", k2), ("v1", v1))}
    return [{name: sh[name][c] for name in sh} for c in range(N_CORES)]


def _gather(results):
    out_bh = np.concatenate([results[c]["out"] for c in range(N_CORES)], axis=0)
    out = out_bh.reshape(B, H, L, D).transpose(0, 2, 1, 3)
    return np.ascontiguousarray(out.astype(np.float32))


def kernel(q1, k1, v1, q2, k2, v2, attn_mask=None, **_unused):
    """Full-input entry point: shards across 8 NeuronCores, returns [B,L,H,D]."""
    in_maps = _shard_inputs(q1, k1, v1, q2, k2)
    nc = _get_program()
    res = run_bass_kernel_spmd(nc, in_maps, list(range(N_CORES))).results
    return _gather(res)


def run_traced(q1, k1, v1, q2, k2, **kwargs):
    """Like kernel() but with NTFF profiling; returns (out, BassKernelResults)."""
    in_maps = _shard_inputs(q1, k1, v1, q2, k2)
    nc = _get_program()
    br = run_bass_kernel_spmd(nc, in_maps, list(range(N_CORES)), trace=True,
                              **kwargs)
    return _gather(br.results), br
